# revision 25
# baseline (speedup 1.0000x reference)
"""Trainium2 Bass kernel for a BasicTransformerBlock (AdaLN + self-attn with
relative position bias + cross-attn + GEGLU FFN), distributed over 8
NeuronCores.

Sharding: core c handles batch b = c//2 and token half h = c%2 (512 of the
1024 tokens of its batch). Token *tiles* (128 tokens each) are permuted
host-side so the core's local tokens are always device tiles 0..3 — this
makes one SPMD program valid for every core; all per-core variation lives in
the input data (including the relative-bias Toeplitz strips).

On-chip dataflow (per core):
  A: load x (full batch, 8 tiles), silu(temb)^T, encoder^T
  B: AdaLN1 over all 8 tiles -> x1 (bf16) -> PE-transpose -> x1^T
  C: self-attention: QKV (head-pair packed), scores^T = K^T-chunks x Q^T with
     rel-bias added on PSUM eviction, exp (no max subtraction; scores are
     small), AV with an appended ones column producing softmax sums for free,
     O-projection from the stacked av^T, residual into fp32 stream
  D: AdaLN2 over local 4 tiles -> x2^T
  E: cross-attention (KV from encoder^T, no bias), residual
  F: eq-LN -> GEGLU FFN (h12 in 512-wide blocks, gelu*a fused on eviction,
     PE-transpose of the gated activations, second matmul), residual, store.

All matmuls run in bf16 with fp32 PSUM accumulation; the residual stream and
all LN statistics stay fp32.
"""

import os
import sys

for _p in ("/opt/trn_rl_repo", "/root/.axon_site/_ro/trn_rl_repo"):
    if os.path.isdir(_p) and _p not in sys.path:
        sys.path.insert(0, _p)

import numpy as np
import ml_dtypes

import concourse.bass as bass
import concourse.mybir as mybir
from concourse import bacc
from concourse.tile import TileContext
from concourse.masks import make_identity

BF = ml_dtypes.bfloat16
F32 = mybir.dt.float32
BF16 = mybir.dt.bfloat16
F32R = mybir.dt.float32r
AF = mybir.ActivationFunctionType
OP = mybir.AluOpType

P = 128
D = 1024
T = 1024
NL = 512          # local tokens per core
H = 16
DH = 64
DI = 4096
G = 4
GS = D // G       # 256
MAXREL = 32
EPS = 1e-5
NT = T // P       # 8 token tiles (full batch)
NLT = NL // P     # 4 local token tiles
NC_ = D // P      # 8 dmodel chunks
REL_W = 896       # width of each Toeplitz strip


# --------------------------------------------------------------------------
# device program
# --------------------------------------------------------------------------

def _ln_normalize(nc, pools, x_ap, out_ap, eps_tile):
    """eq_ln: per-group (G=4, 256 wide) mean/var normalize of one [128, 1024]
    tile. x_ap fp32 in, out_ap (may be bf16) out."""
    stats = pools["stats"]
    for g in range(G):
        xg = x_ap[:, g * GS:(g + 1) * GS]
        st = stats.tile([P, 6], F32, tag="bnst")
        nc.vector.bn_stats(out=st, in_=xg)
        mv = stats.tile([P, 2], F32, tag="bnmv")
        nc.vector.bn_aggr(out=mv, in_=st)
        rs = stats.tile([P, 1], F32, tag="bnrs")
        nc.scalar.activation(out=rs, in_=mv[:, 1:2], func=AF.Sqrt, bias=eps_tile)
        nc.vector.reciprocal(out=rs, in_=rs)
        nc.vector.tensor_scalar(
            out=out_ap[:, g * GS:(g + 1) * GS], in0=xg,
            scalar1=mv[:, 0:1], scalar2=rs, op0=OP.subtract, op1=OP.mult)


def _transpose_tile(nc, pools, src_bf, dst, ident):
    """PE-transpose one [128, 1024-or-512] bf16 tile into dst chunks.
    src_bf: [128, W] bf16; dst callable(c) -> [128, 128] AP destination."""
    W = src_bf.shape[-1]
    for c in range(W // P):
        pt = pools["ps"].tile([P, 512], BF16, tag="ps")
        nc.tensor.transpose(pt[:, :P], src_bf[:, c * P:(c + 1) * P], ident)
        nc.scalar.copy(out=dst(c), in_=pt[:, :P])


def _adaln(nc, pools, tc, n_tiles, x_src, wada_sb, stemb, x1_dst_bf, eps_tile):
    """AdaLN: for each of n_tiles token tiles compute
    ss = silu(temb) @ w_ada^T  (PSUM, 2048 wide in 4 blocks),
    x1 = eq_ln(x) * (1+scale) + shift  -> bf16 into x1_dst_bf[:, t, :]."""
    for t in range(n_tiles):
        # ss matmuls: 4 psum blocks of 512
        ps_blocks = []
        for nb in range(4):
            ps = pools["ps"].tile([P, 512], F32, tag="ps")
            for kc in range(NC_):
                nc.tensor.matmul(
                    ps, stemb[:, kc, t * P:(t + 1) * P],
                    wada_sb[:, kc, nb * 512:(nb + 1) * 512],
                    start=(kc == 0), stop=(kc == NC_ - 1))
            ps_blocks.append(ps)
        # evict: scale1p = 1 + ss[:, :1024], shift = ss[:, 1024:]
        scale1p = pools["work"].tile([P, D], F32, tag="scale1p")
        shift = pools["work"].tile([P, D], F32, tag="shift")
        for nb in range(2):
            nc.scalar.activation(out=scale1p[:, nb * 512:(nb + 1) * 512],
                                 in_=ps_blocks[nb], func=AF.Copy, bias=1.0)
        for nb in range(2):
            nc.scalar.copy(out=shift[:, nb * 512:(nb + 1) * 512],
                           in_=ps_blocks[2 + nb])
        # eq_ln(x)
        n_t = pools["work"].tile([P, D], F32, tag="n_t")
        _ln_normalize(nc, pools, x_src(t), n_t, eps_tile)
        # x1 = n * scale1p + shift (bf16 out); multiply in place
        nc.vector.tensor_tensor(out=n_t, in0=n_t, in1=scale1p, op=OP.mult)
        nc.vector.tensor_tensor(out=x1_dst_bf[:, t, :], in0=n_t, in1=shift,
                                op=OP.add)


def _qkv(nc, pools, q_src, kv_src, wq_sb, wk_sb, wv_sb,
         q_all, k_all, v_all):
    """Project Q^T/K^T per head-pair and V for all pairs at once.
    q_all: [128, 8, NL]; k_all: [128, 8, T]; v_all: [128, NT, 1040]
    (per tt: 8 pair-blocks of 130 = 64 v cols + ones + 64 v cols + ones)."""
    for hp in range(8):
        ps_q = pools["ps"].tile([P, 512], F32, tag="ps")
        for kc in range(NC_):
            nc.tensor.matmul(ps_q, wq_sb[:, kc, hp * P:(hp + 1) * P],
                             q_src[:, kc, :],
                             start=(kc == 0), stop=(kc == NC_ - 1))
        nc.scalar.copy(out=q_all[:, hp, :], in_=ps_q)
        ps_k = pools["psw"].tile([P, 1024], F32, tag="psw")
        for half in range(T // 512):
            for kc in range(NC_):
                nc.tensor.matmul(
                    ps_k[:, half * 512:(half + 1) * 512],
                    wk_sb[:, kc, hp * P:(hp + 1) * P],
                    kv_src[:, kc, half * 512:(half + 1) * 512],
                    start=(kc == 0), stop=(kc == NC_ - 1))
        nc.scalar.copy(out=k_all[:, hp, :], in_=ps_k)
    # V: [tk, hd] via wide matmuls (lhsT = kv chunk reused for both halves)
    for hp in range(8):
        nc.vector.memset(v_all[:, :, hp * 130 + 64:hp * 130 + 65], 1.0)
        nc.vector.memset(v_all[:, :, hp * 130 + 129:hp * 130 + 130], 1.0)
    for tt in range(NT):
        ps_lo = pools["ps"].tile([P, 512], F32, tag="ps")
        ps_hi = pools["ps"].tile([P, 512], F32, tag="ps")
        for kc in range(NC_):
            nc.tensor.matmul(ps_lo, kv_src[:, kc, tt * P:(tt + 1) * P],
                             wv_sb[:, kc, 0:512],
                             start=(kc == 0), stop=(kc == NC_ - 1))
            nc.tensor.matmul(ps_hi, kv_src[:, kc, tt * P:(tt + 1) * P],
                             wv_sb[:, kc, 512:1024],
                             start=(kc == 0), stop=(kc == NC_ - 1))
        for half, psv in ((0, ps_lo), (1, ps_hi)):
            dst = v_all[:, tt, half * 520:(half + 1) * 520].rearrange(
                "p (j c) -> p j c", c=130)
            srcv = psv.rearrange("p (j c) -> p j c", c=128)
            nc.scalar.copy(out=dst[:, :, 0:64], in_=srcv[:, :, 0:64])
            nc.scalar.copy(out=dst[:, :, 65:129], in_=srcv[:, :, 64:128])


def _attention_core(nc, pools, ones_row, q_all, k_all, v_all, wo_sb,
                    exprel_sb, x_res_src, x_out_dst):
    """Software-pipelined over heads: head h's AV/sums work is emitted after
    head h+1's scores/exp so the PE queue always has independent matmuls
    while the softmax elementwise chain of the previous head drains."""
    avT = pools["avT"].tile([P, 8, NL], BF16, tag="avT")

    def emit_scores(hp, hh):
        head = 2 * hp + hh
        es_bf = pools["es"].tile([P, NT, NL], BF16, tag="es")
        erel = exprel_sb(head) if exprel_sb is not None else None
        for bt in range(0, NT, 2):
            ps_s = pools["psw"].tile([P, 1024], F32, tag="psw")
            for j in range(2):
                tt = bt + j
                nc.tensor.matmul(
                    ps_s[:, j * 512:(j + 1) * 512],
                    k_all[hh * 64:(hh + 1) * 64, hp, tt * P:(tt + 1) * P],
                    q_all[hh * 64:(hh + 1) * 64, hp, :],
                    start=True, stop=True)
            if erel is not None:
                esr = pools["cwork"].tile([P, 2, NL], BF16, tag="esr")
                nc.scalar.activation(out=esr, in_=ps_s, func=AF.Exp)
                nc.vector.tensor_tensor(
                    out=es_bf[:, bt:bt + 2, :], in0=esr,
                    in1=erel[:, bt:bt + 2, :], op=OP.mult)
            else:
                nc.scalar.activation(out=es_bf[:, bt:bt + 2, :], in_=ps_s,
                                     func=AF.Exp)
        return es_bf

    def emit_av(hp, hh, es_bf, rsb):
        ps_av = pools["ps"].tile([P, 512], F32, tag="ps")
        for tt in range(NT):
            nc.tensor.matmul(
                ps_av[:65, :],
                v_all[:, tt, hp * 130 + hh * 65:hp * 130 + (hh + 1) * 65],
                es_bf[:, tt, :],
                start=(tt == 0), stop=(tt == NT - 1))
        r0 = hh * 64
        sums_sb = pools["cwork"].tile([1, NL], F32R, tag="sums")
        nc.scalar.copy(out=sums_sb, in_=ps_av[64:65, :])
        ps_b = pools["ps"].tile([P, 512], F32, tag="ps")
        nc.tensor.matmul(ps_b, ones_row, sums_sb, start=True, stop=True)
        nc.vector.reciprocal(out=rsb[r0:r0 + 64, :], in_=ps_b[r0:r0 + 64, :])
        if exprel_sb is None:   # cross-attn: ACT is the busy engine there
            nc.vector.tensor_copy(out=avT[hh * 64:(hh + 1) * 64, hp, :],
                                  in_=ps_av[0:64, :])
        else:
            nc.scalar.copy(out=avT[hh * 64:(hh + 1) * 64, hp, :],
                           in_=ps_av[0:64, :])
        if hh == 1:
            nc.vector.tensor_tensor(out=avT[:, hp, :], in0=avT[:, hp, :],
                                    in1=rsb, op=OP.mult)

    from collections import deque
    pending = deque()
    rsbs = {}
    for hp in range(8):
        rsbs[hp] = pools["cw1"].tile([P, NL], F32, tag="rsb", name=f"rsb{hp}")
        for hh in range(2):
            es_bf = emit_scores(hp, hh)
            if len(pending) >= 2:
                emit_av(*pending.popleft())
            pending.append((hp, hh, es_bf, rsbs[hp]))
    while pending:
        emit_av(*pending.popleft())

    for lt in range(NLT):
        ps_o = pools["psw"].tile([P, 1024], F32, tag="psw")
        for nb in range(2):
            for hp in range(8):
                nc.tensor.matmul(ps_o[:, nb * 512:(nb + 1) * 512],
                                 avT[:, hp, lt * P:(lt + 1) * P],
                                 wo_sb[:, hp, nb * 512:(nb + 1) * 512],
                                 start=(hp == 0), stop=(hp == 7))
        nc.vector.tensor_tensor(out=x_out_dst(lt), in0=ps_o, in1=x_res_src(lt),
                                op=OP.add)


def build_nc(sim_compat=False):
    nc = bacc.Bacc("TRN2", target_bir_lowering=False, debug=False)

    # ---- DRAM parameters (per-core layouts, see prep_inputs) ----
    d_x0 = nc.declare_dram_parameter("x0", [P, NT, D], F32, isOutput=False)
    d_tembt = nc.declare_dram_parameter("tembt", [P, NC_, T], F32, isOutput=False)
    d_enct = nc.declare_dram_parameter("enct", [P, NC_, T], BF16, isOutput=False)
    d_exprel = nc.declare_dram_parameter("exprel", [H, P, NT, NL], BF16,
                                          isOutput=False)
    d_wada1 = nc.declare_dram_parameter("wada1", [P, NC_, 2 * D], BF16, isOutput=False)
    d_wada2 = nc.declare_dram_parameter("wada2", [P, NC_, 2 * D], BF16, isOutput=False)
    d_wq1 = nc.declare_dram_parameter("wq1", [P, NC_, D], BF16, isOutput=False)
    d_wk1 = nc.declare_dram_parameter("wk1", [P, NC_, D], BF16, isOutput=False)
    d_wv1 = nc.declare_dram_parameter("wv1", [P, NC_, D], BF16, isOutput=False)
    d_wo1 = nc.declare_dram_parameter("wo1", [P, NC_, D], BF16, isOutput=False)
    d_wq2 = nc.declare_dram_parameter("wq2", [P, NC_, D], BF16, isOutput=False)
    d_wk2 = nc.declare_dram_parameter("wk2", [P, NC_, D], BF16, isOutput=False)
    d_wv2 = nc.declare_dram_parameter("wv2", [P, NC_, D], BF16, isOutput=False)
    d_wo2 = nc.declare_dram_parameter("wo2", [P, NC_, D], BF16, isOutput=False)
    d_wff1 = nc.declare_dram_parameter("wff1", [P, 16, NC_, 512], BF16, isOutput=False)
    d_wff2 = nc.declare_dram_parameter("wff2", [P, 32, D], BF16, isOutput=False)
    d_out = nc.declare_dram_parameter("out", [P, NLT, D], F32, isOutput=True)

    from contextlib import ExitStack
    with TileContext(nc) as tc, ExitStack() as glob:
        pools = {}
        const = glob.enter_context(tc.tile_pool(name="const", bufs=1))
        pools["ps"] = glob.enter_context(tc.tile_pool(name="ps", bufs=4, space="PSUM"))
        pools["psw"] = glob.enter_context(tc.tile_pool(name="psw", bufs=2, space="PSUM"))
        pools["stats"] = glob.enter_context(tc.tile_pool(name="stats", bufs=4))
        p_xB = glob.enter_context(tc.tile_pool(name="xB_pool", bufs=1))

        ident = const.tile([P, P], BF16)
        make_identity(nc, ident)
        eps_tile = const.tile([P, 1], F32)
        nc.vector.memset(eps_tile, EPS)
        ones_row_f = const.tile([1, P], F32)
        nc.vector.memset(ones_row_f, 1.0)
        ones_row = const.tile([1, P], F32R)
        nc.scalar.copy(out=ones_row, in_=ones_row_f)
        p_xB = glob.enter_context(tc.tile_pool(name="xB_pool", bufs=1))
        xB = p_xB.tile([P, NLT, D], F32)
        xB = p_xB.tile([P, NLT, D], F32)

        xA_stk = ExitStack()         # -> closes after E
        p_xA = xA_stk.enter_context(tc.tile_pool(name="xA_pool", bufs=1))
        xA = p_xA.tile([P, NLT, D], F32)

        stemb_stk = ExitStack()      # -> closes after E
        p_stemb = stemb_stk.enter_context(tc.tile_pool(name="stemb", bufs=1))
        stemb = p_stemb.tile([P, NC_, T], BF16)

        mid1 = ExitStack()           # x1t, xbase: -> close after C
        p_xbase = mid1.enter_context(tc.tile_pool(name="xbase_pool", bufs=1))
        p_x1t = mid1.enter_context(tc.tile_pool(name="x1t_pool", bufs=1))
        x1t = p_x1t.tile([P, NC_, T], BF16)
        xbase = p_xbase.tile([P, NLT, D], F32)

        # ---------------- stage A+B: loads, silu, AdaLN1, transpose --------
        stg = ExitStack()
        pools["work"] = stg.enter_context(tc.tile_pool(name="awork", bufs=2))
        p_wada1 = stg.enter_context(tc.tile_pool(name="wada1_pool", bufs=1))
        p_x1s = stg.enter_context(tc.tile_pool(name="x1_stage", bufs=1))
        nc.sync.dma_start(out=xbase, in_=d_x0[:, 0:NLT, :])
        for kc in range(NC_):
            tmb = pools["work"].tile([P, T], F32, tag="scale1p")
            nc.sync.dma_start(out=tmb, in_=d_tembt[:, kc, :])
            sg = pools["work"].tile([P, T], F32, tag="shift")
            nc.scalar.activation(out=sg, in_=tmb, func=AF.Sigmoid)
            nc.vector.tensor_tensor(out=stemb[:, kc, :], in0=tmb, in1=sg,
                                    op=OP.mult)
        wada1 = p_wada1.tile([P, NC_, 2 * D], BF16)
        for kc in range(NC_):
            nc.sync.dma_start(out=wada1[:, kc, :], in_=d_wada1[:, kc, :])

        def x0_src(t):
            xt = pools["work"].tile([P, D], F32, tag="x0t")
            nc.sync.dma_start(out=xt, in_=d_x0[:, t, :])
            return xt

        x1_tiles = p_x1s.tile([P, NT, D], BF16)
        _adaln(nc, pools, tc, NT, x0_src, wada1, stemb,
               x1_tiles, eps_tile)
        for t in range(NT):
            _transpose_tile(
                nc, pools, x1_tiles[:, t, :],
                lambda c, t=t: x1t[:, c, t * P:(t + 1) * P], ident)
        stg.close()

        # ---------------- stage C: self-attention --------------------------
        qkv_stk = ExitStack()
        p_qkv = qkv_stk.enter_context(tc.tile_pool(name="qkv1", bufs=1))
        q_all = p_qkv.tile([P, 8, NL], BF16, tag="q_all")
        k_all = p_qkv.tile([P, 8, T], BF16, tag="k_all")
        v_all = p_qkv.tile([P, NT, 1040], BF16, tag="v_all")
        stg = ExitStack()
        p_w1 = stg.enter_context(tc.tile_pool(name="wqkv1", bufs=1))
        wq1 = p_w1.tile([P, NC_, D], BF16, tag="wq")
        wk1 = p_w1.tile([P, NC_, D], BF16, tag="wk")
        wv1 = p_w1.tile([P, NC_, D], BF16, tag="wv")
        for kc in range(NC_):
            nc.sync.dma_start(out=wq1[:, kc, :], in_=d_wq1[:, kc, :])
        for kc in range(NC_):
            nc.sync.dma_start(out=wk1[:, kc, :], in_=d_wk1[:, kc, :])
        for kc in range(NC_):
            nc.sync.dma_start(out=wv1[:, kc, :], in_=d_wv1[:, kc, :])
        _qkv(nc, pools, x1t[:, :, 0:NL], x1t, wq1, wk1, wv1,
             q_all, k_all, v_all)
        stg.close()

        stg = ExitStack()
        p_wo1 = stg.enter_context(tc.tile_pool(name="wo1_pool", bufs=1))
        p_rel = stg.enter_context(tc.tile_pool(name="rel_pool", bufs=2))
        pools["es"] = stg.enter_context(tc.tile_pool(name="es_pool", bufs=3))
        pools["cwork"] = stg.enter_context(tc.tile_pool(name="cwork", bufs=2))
        pools["cw1"] = stg.enter_context(tc.tile_pool(name="cw1", bufs=3))
        pools["avT"] = stg.enter_context(tc.tile_pool(name="avT_pool", bufs=1))
        wo1 = p_wo1.tile([P, NC_, D], BF16)
        for kc in range(NC_):
            nc.sync.dma_start(out=wo1[:, kc, :], in_=d_wo1[:, kc, :])

        def exprel_sb(head):
            er = p_rel.tile([P, NT, NL], BF16, tag="exprel")
            nc.sync.dma_start(out=er, in_=d_exprel[head])
            return er

        _attention_core(nc, pools, ones_row, q_all, k_all, v_all, wo1,
                        exprel_sb,
                        x_res_src=lambda lt: xbase[:, lt, :],
                        x_out_dst=lambda lt: xA[:, lt, :])
        stg.close()
        qkv_stk.close()
        mid1.close()

        # ---------------- stage D: AdaLN2 + transpose ----------------------
        x2t_stk = ExitStack()        # -> close after E
        p_x2t = x2t_stk.enter_context(tc.tile_pool(name="x2t_pool", bufs=1))
        x2t = p_x2t.tile([P, NC_, NL], BF16)
        stg = ExitStack()
        pools["work"] = stg.enter_context(tc.tile_pool(name="dwork", bufs=2))
        p_wada2 = stg.enter_context(tc.tile_pool(name="wada2_pool", bufs=1))
        wada2 = p_wada2.tile([P, NC_, 2 * D], BF16)
        for kc in range(NC_):
            nc.sync.dma_start(out=wada2[:, kc, :], in_=d_wada2[:, kc, :])
        p_x2s = stg.enter_context(tc.tile_pool(name="x2_stage", bufs=1))
        x2_tiles = p_x2s.tile([P, NLT, D], BF16)
        _adaln(nc, pools, tc, NLT, lambda t: xA[:, t, :], wada2,
               stemb, x2_tiles, eps_tile)
        for t in range(NLT):
            _transpose_tile(
                nc, pools, x2_tiles[:, t, :],
                lambda c, t=t: x2t[:, c, t * P:(t + 1) * P], ident)
        stg.close()

        # ---------------- stage E: cross-attention -------------------------
        qkv_stk = ExitStack()
        p_qkv2 = qkv_stk.enter_context(tc.tile_pool(name="qkv2", bufs=1))
        q2_all = p_qkv2.tile([P, 8, NL], BF16, tag="q_all")
        k2_all = p_qkv2.tile([P, 8, T], BF16, tag="k_all")
        v2_all = p_qkv2.tile([P, NT, 1040], BF16, tag="v_all")
        stg = ExitStack()
        p_enc = stg.enter_context(tc.tile_pool(name="enc_pool", bufs=1))
        p_w2 = stg.enter_context(tc.tile_pool(name="wqkv2", bufs=1))
        enc = p_enc.tile([P, NC_, T], BF16)
        for kc in range(NC_):
            nc.sync.dma_start(out=enc[:, kc, :], in_=d_enct[:, kc, :])
        wq2 = p_w2.tile([P, NC_, D], BF16, tag="wq")
        wk2 = p_w2.tile([P, NC_, D], BF16, tag="wk")
        wv2 = p_w2.tile([P, NC_, D], BF16, tag="wv")
        for kc in range(NC_):
            nc.sync.dma_start(out=wq2[:, kc, :], in_=d_wq2[:, kc, :])
        for kc in range(NC_):
            nc.sync.dma_start(out=wk2[:, kc, :], in_=d_wk2[:, kc, :])
        for kc in range(NC_):
            nc.sync.dma_start(out=wv2[:, kc, :], in_=d_wv2[:, kc, :])
        _qkv(nc, pools, x2t, enc, wq2, wk2, wv2, q2_all, k2_all, v2_all)
        stg.close()

        stg = ExitStack()
        p_wo2 = stg.enter_context(tc.tile_pool(name="wo2_pool", bufs=1))
        pools["es"] = stg.enter_context(tc.tile_pool(name="es2_pool", bufs=3))
        pools["cwork"] = stg.enter_context(tc.tile_pool(name="cwork2", bufs=2))
        pools["cw1"] = stg.enter_context(tc.tile_pool(name="cw12", bufs=3))
        pools["avT"] = stg.enter_context(tc.tile_pool(name="avT2_pool", bufs=1))
        wo2 = p_wo2.tile([P, NC_, D], BF16)
        for kc in range(NC_):
            nc.sync.dma_start(out=wo2[:, kc, :], in_=d_wo2[:, kc, :])

        _attention_core(nc, pools, ones_row, q2_all, k2_all, v2_all, wo2,
                        None,
                        x_res_src=lambda lt: xA[:, lt, :],
                        x_out_dst=lambda lt: xB[:, lt, :])
        stg.close()
        qkv_stk.close()
        x2t_stk.close()
        stemb_stk.close()
        xA_stk.close()

        # ---------------- stage F: eq-LN + GEGLU FFN -----------------------
        stg = ExitStack()
        p_n3t = stg.enter_context(tc.tile_pool(name="n3t_pool", bufs=1))
        p_gT = stg.enter_context(tc.tile_pool(name="gatedT_pool", bufs=1))
        p_wff1 = stg.enter_context(tc.tile_pool(name="wff1_pool", bufs=2))
        p_wff2 = stg.enter_context(tc.tile_pool(name="wff2_pool", bufs=1))
        p_fw = stg.enter_context(tc.tile_pool(name="fwork", bufs=3))
        n3t = p_n3t.tile([P, NC_, NL], BF16)
        for t in range(NLT):
            n3 = p_fw.tile([P, D], BF16, tag="n3")
            _ln_normalize(nc, pools, xB[:, t, :], n3, eps_tile)
            _transpose_tile(
                nc, pools, n3,
                lambda c, t=t: n3t[:, c, t * P:(t + 1) * P],
                ident)
        gatedT = p_gT.tile([P, 32, NL], BF16)
        wff2 = p_wff2.tile([P, 32, D], BF16)
        for dc in range(32):
            nc.sync.dma_start(out=wff2[:, dc, :], in_=d_wff2[:, dc, :])
        pend_t = None
        for nbh in range(8):
            wa = p_wff1.tile([P, NC_, 512], BF16, tag="wff1")
            wg = p_wff1.tile([P, NC_, 512], BF16, tag="wff1g")
            nc.sync.dma_start(out=wa, in_=d_wff1[:, nbh])
            nc.sync.dma_start(out=wg, in_=d_wff1[:, 8 + nbh])
            for lt in range(NLT):
                ps_a = pools["ps"].tile([P, 512], F32, tag="ps")
                ps_g = pools["ps"].tile([P, 512], F32, tag="ps")
                for kc in range(NC_):
                    nc.tensor.matmul(
                        ps_a, n3t[:, kc, lt * P:(lt + 1) * P],
                        wa[:, kc, :],
                        start=(kc == 0), stop=(kc == NC_ - 1))
                    nc.tensor.matmul(
                        ps_g, n3t[:, kc, lt * P:(lt + 1) * P],
                        wg[:, kc, :],
                        start=(kc == 0), stop=(kc == NC_ - 1))
                g_bf = p_fw.tile([P, 512], BF16, tag="g_bf")
                ga_bf = p_fw.tile([P, 512], BF16, tag="ga_bf")
                if sim_compat:
                    # CoreSim has no Gelu table: use x*sigmoid(1.702x) and
                    # compare against the same formula host-side.
                    graw = p_fw.tile([P, 512], F32, tag="graw")
                    nc.scalar.copy(out=graw, in_=ps_g)
                    nc.scalar.activation(out=g_bf, in_=ps_g, func=AF.Sigmoid,
                                         scale=1.702)
                    nc.vector.tensor_tensor(out=g_bf, in0=g_bf, in1=graw,
                                            op=OP.mult)
                else:
                    nc.scalar.activation(out=g_bf, in_=ps_g, func=AF.Gelu)
                nc.vector.tensor_tensor(out=ga_bf, in0=ps_a, in1=g_bf,
                                        op=OP.mult)
                if pend_t is not None:
                    pga, pnbh, plt = pend_t
                    _transpose_tile(
                        nc, pools, pga,
                        lambda c, nbh=pnbh, lt=plt: gatedT[
                            :, nbh * 4 + c, lt * P:(lt + 1) * P],
                        ident)
                pend_t = (ga_bf, nbh, lt)
        pga, pnbh, plt = pend_t
        _transpose_tile(
            nc, pools, pga,
            lambda c, nbh=pnbh, lt=plt: gatedT[
                :, nbh * 4 + c, lt * P:(lt + 1) * P],
            ident)
        for lt in range(NLT):
            ps2 = pools["psw"].tile([P, 1024], F32, tag="psw")
            for dc in range(32):
                for nb in range(2):
                    nc.tensor.matmul(
                        ps2[:, nb * 512:(nb + 1) * 512],
                        gatedT[:, dc, lt * P:(lt + 1) * P],
                        wff2[:, dc, nb * 512:(nb + 1) * 512],
                        start=(dc == 0), stop=(dc == 31))
            o_sb = p_fw.tile([P, D], F32, tag="o_sb")
            nc.vector.tensor_tensor(out=o_sb, in0=ps2, in1=xB[:, lt, :],
                                    op=OP.add)
            nc.sync.dma_start(out=d_out[:, lt, :], in_=o_sb)
        stg.close()
    return nc


# --------------------------------------------------------------------------
# host-side input preparation
# --------------------------------------------------------------------------

def _chunk_w(w_t, n_chunks):
    """(D_in, N) -> [128, n_chunks, N] with [p, c, n] = w_t[c*128+p, n]."""
    D_in, N = w_t.shape
    return np.ascontiguousarray(
        w_t.reshape(n_chunks, P, N).transpose(1, 0, 2)).astype(BF)


def prep_core_inputs(core, inputs):
    b, h = core // 2, core % 2
    hs = inputs["hidden_states"][b]          # (1024, 1024) f32
    enc = inputs["encoder_hidden_states"][b]
    temb = inputs["temb"][b * T:(b + 1) * T]

    perm = [(tt + 4 * h) % 8 for tt in range(NT)]

    x0 = hs.reshape(NT, P, D)[perm].transpose(1, 0, 2)  # [p, tt, d]
    x0 = np.ascontiguousarray(x0).astype(np.float32)

    temb_perm = temb.reshape(NT, P, D)[perm].reshape(T, D)  # permuted tokens
    tembt = np.ascontiguousarray(
        temb_perm.T.reshape(NC_, P, T).transpose(1, 0, 2)).astype(np.float32)

    enct = np.ascontiguousarray(
        enc.T.reshape(NC_, P, T).transpose(1, 0, 2)).astype(BF)

    # exp of the relative bias, prearranged per (head, k-tile):
    # exprel[head, p, tt, u] = exp(bias(tq_global = h*512+u,
    #                               tk_global = gt(tt)*128 + p))
    rel = inputs["rel_bias"]                  # (16, 65) f32
    qg = h * NL
    uu = np.arange(NL)[None, None, :]
    pp2 = np.arange(P)[:, None, None]
    k0g = (np.array(perm) * P)[None, :, None]
    delta = np.clip((qg + uu) - (k0g + pp2), -MAXREL, MAXREL) + MAXREL
    exprel = np.exp(rel[:, delta]).astype(BF)  # (16, 128, 8, 512)

    out = {
        "x0": x0, "tembt": tembt, "enct": enct,
        "exprel": np.ascontiguousarray(exprel),
        "wada1": _chunk_w(inputs["w_ada1"].T, NC_),
        "wada2": _chunk_w(inputs["w_ada2"].T, NC_),
        "wq1": _chunk_w(inputs["wq1"].T / (DH ** 0.5), NC_),
        "wk1": _chunk_w(inputs["wk1"].T, NC_),
        "wv1": _chunk_w(inputs["wv1"].T, NC_),
        "wo1": _chunk_w(inputs["wo1"].T, NC_),
        "wq2": _chunk_w(inputs["wq2"].T / (DH ** 0.5), NC_),
        "wk2": _chunk_w(inputs["wk2"].T, NC_),
        "wv2": _chunk_w(inputs["wv2"].T, NC_),
        "wo2": _chunk_w(inputs["wo2"].T, NC_),
        # wff1: [p, nb(16), c(8), n(512)] = w_ff1[nb*512+n, c*128+p]
        "wff1": np.ascontiguousarray(
            inputs["w_ff1"].reshape(16, 512, NC_, P)
            .transpose(3, 0, 2, 1)).astype(BF),
        "wff2": _chunk_w(inputs["w_ff2"].T, 32),
    }
    return out


def check_zero_biases(inputs):
    for k in ("b_ada1", "b_ada2", "bo1", "bo2", "b_ff1", "b_ff2"):
        if np.any(np.asarray(inputs[k])):
            raise NotImplementedError(
                f"bias {k} is nonzero; this kernel build assumes zero biases")


_NC_CACHE = []


def kernel(**inputs):
    check_zero_biases(inputs)
    from concourse.bass_utils import run_bass_kernel_spmd
    if not _NC_CACHE:
        nc = build_nc()
        nc.compile()
        _NC_CACHE.append(nc)
    nc = _NC_CACHE[0]
    in_maps = [prep_core_inputs(c, inputs) for c in range(8)]
    res = run_bass_kernel_spmd(nc, in_maps, list(range(8)))
    B = inputs["hidden_states"].shape[0]
    out = np.empty((B, T, D), np.float32)
    for c in range(8):
        b, h = c // 2, c % 2
        o = res.results[c]["out"]            # [128, 4, 1024]
        out[b, h * NL:(h + 1) * NL] = o.transpose(1, 0, 2).reshape(NL, D)
    return out
